# revision 3
# baseline (speedup 1.0000x reference)
"""Conv2d(128->256, 3x3, pad 1) with LoRA (rank 8) — Trainium2 Bass kernel.

Strategy (v3):
  - Data-parallel over batch: 16 images -> 2 per core x 8 cores. Conv weights
    and LoRA A/B replicated.
  - LoRA folds into the conv weight on device (conv is linear in weights):
        W_eff = W + (alpha/rank) * (B @ A).reshape(C_OUT, C_IN, 3, 3)
    via 9 tiny PE matmuls (K=8) + fused DVE scalar_tensor_tensor adds.
  - The 3x3 conv itself = 9 shifted matmuls accumulating in PSUM:
        out[co, pix] += W_eff[co, :, kh, kw]^T @ x_shift[ci, pix]
    with K = C_IN = 128 (partition dim), M = 128 (co block), N = 512
    (8 image rows x 64 cols) in bf16 — 1 col/cycle = full PE rate; the
    288-matmul stream is the bf16 roofline (~62 us warm).
  - All tensor I/O is bf16 (host does the identical RNE rounding the DVE
    used to do; output is written bf16 and upcast on host). Halves DMA
    traffic, removes every DVE cast.
  - Head: A/B land first (one bundled DMA) -> LoRA matmuls double as PE
    warm-up; wt arrives in quarters on both HW queues; the conv is emitted
    in k-minor 4-row-group waves so the in-order PE queue chases the weff
    folds without stalling.
  - x image 1 + bias stream via the gpsimd SWDGE queue, keeping both HWDGE
    queues free for x0/wt in and output tiles out.
"""

import numpy as np
import ml_dtypes

import concourse.bass as bass
import concourse.tile as tile
from concourse.tile import add_dep_helper
from concourse import bacc, mybir
from concourse.bass_utils import run_bass_kernel_spmd

N_CORES = 8
B, C_IN, H, W_DIM = 16, 128, 64, 64
C_OUT = 256
RANK = 8
SCALING = 2.0  # alpha/rank = 16/8
HP, WP = H + 2, W_DIM + 2  # zero-padded image dims
B_LOC = B // N_CORES  # images per core
NPIX = H * W_DIM  # 4096
ROWS_PER_TILE = 8  # output rows per matmul group -> N = 8*64 = 512
N_RG = H // ROWS_PER_TILE  # 8 row groups

F32 = mybir.dt.float32
BF16 = mybir.dt.bfloat16
IDENT = mybir.ActivationFunctionType.Identity
BF16_NP = ml_dtypes.bfloat16


def _build_nc():
    nc = bacc.Bacc(
        "TRN2",
        target_bir_lowering=False,
        debug=False,
        num_devices=N_CORES,
    )

    xp = nc.dram_tensor("xp", [B_LOC, C_IN, HP * WP], BF16, kind="ExternalInput").ap()
    wt = nc.dram_tensor("wt", [C_IN, 9 * C_OUT], BF16, kind="ExternalInput").ap()
    # at and bt bundled: [8, 9*128 | 256] -> one DMA, one completion
    ab = nc.dram_tensor("ab", [RANK, 9 * C_IN + C_OUT], BF16, kind="ExternalInput").ap()
    bv = nc.dram_tensor("bv", [128, 2], F32, kind="ExternalInput").ap()
    out = nc.dram_tensor("out", [B_LOC, C_OUT, NPIX], BF16, kind="ExternalOutput").ap()

    with tile.TileContext(nc) as tc:
        with (
            tc.tile_pool(name="persist", bufs=1) as persist,
            tc.tile_pool(name="outp", bufs=6) as outp,
            tc.tile_pool(name="psum", bufs=8, space="PSUM") as psum,
        ):
            # --- persistent SBUF tiles (all bf16 straight off DMA) ----------
            x_sb = [
                persist.tile([C_IN, HP * WP], BF16, name=f"x_sb{i}")
                for i in range(B_LOC)
            ]
            wt_sb = persist.tile([C_IN, 9 * C_OUT], BF16, name="wt_sb")
            weff = persist.tile([C_IN, 9 * C_OUT], BF16, name="weff")
            ab_sb = persist.tile([RANK, 9 * C_IN + C_OUT], BF16, name="ab_sb")
            b_sb = persist.tile([128, 2], F32, name="b_sb")
            warm_sb = persist.tile([128, 512], BF16, name="warm_sb")

            # --- input DMAs ------------------------------------------------
            # Queue FIFO order = priority order; each DMA_DIRECT2D costs
            # ~0.65us of issue time on its queue engine and completion sems
            # lag the data by ~1.5-2us (HBM write receipt). Critical path to
            # the first conv matmul: ab -> LoRA MMs -> (with wt q0) weff
            # fold 0; x0 rows chase the first wave's row-groups.
            qs = [nc.sync, nc.scalar]
            wq = (9 * C_OUT) // 4  # wt quarter = 576 cols
            # x0 chunks by padded-image columns (row r lives at cols 66r..):
            xc = [0, 10 * WP, 26 * WP, 42 * WP, 58 * WP, HP * WP]

            def xdma(eng, i, c):
                eng.dma_start(x_sb[i][:, xc[c] : xc[c + 1]], xp[i, :, xc[c] : xc[c + 1]])

            # sync: wt q0, x0c1, wt q1, x0c2, wt q3
            # scalar: ab, x0c0, wt q2, x0c3, x0c4
            nc.sync.dma_start(wt_sb[:, 0:wq], wt[:, 0:wq])
            nc.scalar.dma_start(ab_sb[:], ab)
            nc.scalar.dma_start(x_sb[0][:, : xc[1]], xp[0, :, : xc[1]])
            xdma(nc.sync, 0, 1)
            nc.sync.dma_start(wt_sb[:, wq : 2 * wq], wt[:, wq : 2 * wq])
            nc.scalar.dma_start(wt_sb[:, 2 * wq : 3 * wq], wt[:, 2 * wq : 3 * wq])
            xdma(nc.sync, 0, 2)
            xdma(nc.scalar, 0, 3)
            nc.sync.dma_start(wt_sb[:, 3 * wq :], wt[:, 3 * wq :])
            xdma(nc.scalar, 0, 4)
            # bias + image 1 via SWDGE: keeps HW queues free for outputs;
            # x1 is only needed ~35us in.
            nc.gpsimd.dma_start(b_sb[:], bv)
            h1 = (HP * WP) // 2
            nc.gpsimd.dma_start(x_sb[1][:, :h1], xp[1, :, :h1])
            nc.gpsimd.dma_start(x_sb[1][:, h1:], xp[1, :, h1:])

            # --- PE warm-up ------------------------------------------------
            # The HAM clock gate holds the PE at 1.2 GHz until ~3.4us of
            # sustained busy. DVE is idle at the head, so it memsets the
            # warm tile immediately and the dummy matmuls (no DMA deps)
            # start the busy window at ~first-issue time.
            nc.vector.memset(warm_sb[:], 0.0)
            lps = [
                psum.tile([128, 512], F32, tag="lps", bufs=3, name=f"lps{j}")
                for j in range(5)
            ]
            for _ in range(3):
                nc.tensor.matmul(
                    lps[0][:], warm_sb[:, :128], warm_sb[:], start=True, stop=True
                )

            # --- fold LoRA into the conv weight ----------------------------
            # lps[j][:, (k%2)*256:...] = (A_k)^T @ B^T  for k = 2j, 2j+1
            # weff[:, k*256+co] = wt[:, k*256+co] + 2 * lps[...]
            # The 9 K=8 matmuls also extend the PE warm-up.
            for k in range(9):
                nc.tensor.matmul(
                    lps[k // 2][:, (k % 2) * 256 : (k % 2) * 256 + 256],
                    ab_sb[:, k * 128 : (k + 1) * 128],
                    ab_sb[:, 9 * C_IN :],
                    start=True,
                    stop=True,
                )

            # DVE stream is FIFO and the scheduler's DMA-latency model is
            # optimistic: chain the weff folds in k order so conv wave 0
            # starts progressively off fold #0.
            def chain(inst, prev, why):
                if prev is not None:
                    add_dep_helper(inst.ins, prev.ins, sync=False, reason=why)
                return inst

            link = None
            for j in range(5):
                w = 512 if j < 4 else 256
                link = chain(
                    nc.vector.scalar_tensor_tensor(
                        weff[:, j * 512 : j * 512 + w],
                        lps[j][:, :w],
                        SCALING,
                        wt_sb[:, j * 512 : j * 512 + w],
                        op0=mybir.AluOpType.mult,
                        op1=mybir.AluOpType.add,
                    ),
                    link,
                    "weff fold k order",
                )

            # --- the conv: 9 accumulating shift-matmuls per output tile ----
            # Emitted k-minor in 4-row-group waves: the in-order PE queue
            # then needs weff fold j only ~8 matmuls after fold j-1, so it
            # chases the DVE chain without stalling, and each weight is
            # loaded once per wave instead of once per tile.
            for img in range(B_LOC):
                x_r = x_sb[img][:].rearrange("p (h w) -> p h w", w=WP)
                for cb in range(2):
                    for wv in range(2):
                        rgs = [wv * 4 + j for j in range(4)]
                        ps = {
                            rg: psum.tile(
                                [128, 512], F32, tag="ps", bufs=5,
                                name=f"ps{img}_{cb}_{rg}",
                            )
                            for rg in rgs
                        }
                        for k in range(9):
                            dh, dw = k // 3 - 1, k % 3 - 1
                            lhsT = weff[
                                :, k * 256 + cb * 128 : k * 256 + cb * 128 + 128
                            ]
                            for rg in rgs:
                                h0 = rg * ROWS_PER_TILE
                                rhs = x_r[
                                    :,
                                    h0 + 1 + dh : h0 + 1 + dh + ROWS_PER_TILE,
                                    1 + dw : 65 + dw,
                                ]
                                nc.tensor.matmul(
                                    ps[rg][:],
                                    lhsT,
                                    rhs,
                                    start=(k == 0),
                                    stop=(k == 8),
                                )
                        for rg in rgs:
                            o = outp.tile(
                                [128, 512], BF16, tag="o", name=f"o{img}_{cb}_{rg}"
                            )
                            ti = (img * 2 + cb) * N_RG + rg
                            # Alternate the PSUM->SBUF bias-add between ACT
                            # and DVE so neither engine limits PSUM drain.
                            if ti % 2 == 0:
                                nc.scalar.activation(
                                    o[:], ps[rg][:], IDENT, bias=b_sb[:, cb : cb + 1]
                                )
                            else:
                                nc.vector.tensor_scalar_add(
                                    o[:], ps[rg][:], b_sb[:, cb : cb + 1]
                                )
                            dst = out[
                                img,
                                cb * 128 : (cb + 1) * 128,
                                rg * 512 : (rg + 1) * 512,
                            ]
                            if ti >= 30:
                                # split the final tiles across both queues
                                # to shorten the drain tail
                                qs[0].dma_start(dst[:, :256], o[:, :256])
                                qs[1].dma_start(dst[:, 256:], o[:, 256:])
                            else:
                                qs[ti % 2].dma_start(dst, o[:])

    nc.compile()
    return nc


_NC_CACHE = None


def _get_nc():
    global _NC_CACHE
    if _NC_CACHE is None:
        _NC_CACHE = _build_nc()
    return _NC_CACHE


def _host_prep(x, W, b, lora_A, lora_B):
    """Layout + bf16 rounding on host (identical RNE rounding to the DVE
    casts the kernel previously performed on device); no other arithmetic."""
    x = np.ascontiguousarray(x, dtype=np.float32)
    xp_all = np.zeros((B, C_IN, HP, WP), dtype=np.float32)
    xp_all[:, :, 1 : H + 1, 1 : W_DIM + 1] = x
    xp_all = xp_all.reshape(B, C_IN, HP * WP).astype(BF16_NP)

    # [co, ci, kh, kw] -> [ci, k, co]
    wt = (
        np.ascontiguousarray(
            np.asarray(W, dtype=np.float32).reshape(C_OUT, C_IN, 9).transpose(1, 2, 0)
        )
        .reshape(C_IN, 9 * C_OUT)
        .astype(BF16_NP)
    )
    # lora_A [r, ci*9+k] -> [r, k, ci]; lora_B [co, r] -> [r, co]; bundled
    at = np.asarray(lora_A, dtype=np.float32).reshape(RANK, C_IN, 9).transpose(0, 2, 1)
    bt = np.asarray(lora_B, dtype=np.float32).T
    ab = np.concatenate(
        [at.reshape(RANK, 9 * C_IN), bt], axis=1
    ).astype(BF16_NP)
    ab = np.ascontiguousarray(ab)
    # [256] -> [128, 2]: bv[p, cb] = b[cb*128 + p]
    bv = np.ascontiguousarray(np.asarray(b, dtype=np.float32).reshape(2, 128).T)
    return xp_all, wt, ab, bv


def run(x, W, b, lora_A, lora_B, trace=False):
    """Run the kernel on 8 cores; returns (full_output, BassKernelResults)."""
    xp_all, wt, ab, bv = _host_prep(x, W, b, lora_A, lora_B)
    nc = _get_nc()
    in_maps = []
    for c in range(N_CORES):
        in_maps.append(
            {
                "xp": np.ascontiguousarray(xp_all[c * B_LOC : (c + 1) * B_LOC]),
                "wt": wt,
                "ab": ab,
                "bv": bv,
            }
        )
    res = run_bass_kernel_spmd(
        nc, in_maps, core_ids=list(range(N_CORES)), trace=trace
    )
    out = np.concatenate(
        [r["out"].astype(np.float32) for r in res.results], axis=0
    )
    return out.reshape(B, C_OUT, H, W_DIM), res


def kernel(x, W, b, lora_A, lora_B):
    out, _ = run(x, W, b, lora_A, lora_B, trace=False)
    return out


# revision 7
# speedup vs baseline: 1.0014x; 1.0014x over previous
"""Conv2d(128->256, 3x3, pad 1) with LoRA (rank 8) — Trainium2 Bass kernel.

Strategy (v3):
  - Data-parallel over batch: 16 images -> 2 per core x 8 cores. Conv weights
    and LoRA A/B replicated.
  - LoRA folds into the conv weight on device (conv is linear in weights):
        W_eff = W + (alpha/rank) * (B @ A).reshape(C_OUT, C_IN, 3, 3)
    via 9 tiny PE matmuls (K=8) + fused DVE scalar_tensor_tensor adds.
  - The 3x3 conv itself = 9 shifted matmuls accumulating in PSUM:
        out[co, pix] += W_eff[co, :, kh, kw]^T @ x_shift[ci, pix]
    with K = C_IN = 128 (partition dim), M = 128 (co block), N = 512
    (8 image rows x 64 cols) in bf16 — 1 col/cycle = full PE rate; the
    288-matmul stream is the bf16 roofline (~62 us warm).
  - All tensor I/O is bf16 (host does the identical RNE rounding the DVE
    used to do; output is written bf16 and upcast on host). Halves DMA
    traffic, removes every DVE cast.
  - Head: A/B land first (one bundled DMA) -> LoRA matmuls double as PE
    warm-up; wt arrives in quarters on both HW queues; the conv is emitted
    in k-minor 4-row-group waves so the in-order PE queue chases the weff
    folds without stalling.
  - x image 1 + bias stream via the gpsimd SWDGE queue, keeping both HWDGE
    queues free for x0/wt in and output tiles out.
"""

import numpy as np
import ml_dtypes

import concourse.bass as bass
import concourse.tile as tile
from concourse.tile import add_dep_helper
from concourse import bacc, mybir
from concourse.bass_utils import run_bass_kernel_spmd

N_CORES = 8
B, C_IN, H, W_DIM = 16, 128, 64, 64
C_OUT = 256
RANK = 8
SCALING = 2.0  # alpha/rank = 16/8
HP, WP = H + 2, W_DIM + 2  # zero-padded image dims
B_LOC = B // N_CORES  # images per core
NPIX = H * W_DIM  # 4096
ROWS_PER_TILE = 8  # output rows per matmul group -> N = 8*64 = 512
N_RG = H // ROWS_PER_TILE  # 8 row groups

F32 = mybir.dt.float32
BF16 = mybir.dt.bfloat16
IDENT = mybir.ActivationFunctionType.Identity
BF16_NP = ml_dtypes.bfloat16


def _build_nc():
    nc = bacc.Bacc(
        "TRN2",
        target_bir_lowering=False,
        debug=False,
        num_devices=N_CORES,
    )

    xp = nc.dram_tensor("xp", [B_LOC, C_IN, HP * WP], BF16, kind="ExternalInput").ap()
    wt = nc.dram_tensor("wt", [C_IN, 9 * C_OUT], BF16, kind="ExternalInput").ap()
    # at and bt bundled: [8, 9*128 | 256] -> one DMA, one completion
    ab = nc.dram_tensor("ab", [RANK, 9 * C_IN + C_OUT], BF16, kind="ExternalInput").ap()
    bv = nc.dram_tensor("bv", [128, 2], F32, kind="ExternalInput").ap()
    out = nc.dram_tensor("out", [B_LOC, C_OUT, NPIX], BF16, kind="ExternalOutput").ap()

    with tile.TileContext(nc) as tc:
        with (
            tc.tile_pool(name="persist", bufs=1) as persist,
            tc.tile_pool(name="outp", bufs=6) as outp,
            tc.tile_pool(name="psum", bufs=8, space="PSUM") as psum,
        ):
            # --- persistent SBUF tiles (all bf16 straight off DMA) ----------
            x_sb = [
                persist.tile([C_IN, HP * WP], BF16, name=f"x_sb{i}")
                for i in range(B_LOC)
            ]
            wt_sb = persist.tile([C_IN, 9 * C_OUT], BF16, name="wt_sb")
            weff = persist.tile([C_IN, 9 * C_OUT], BF16, name="weff")
            ab_sb = persist.tile([RANK, 9 * C_IN + C_OUT], BF16, name="ab_sb")
            b_sb = persist.tile([128, 2], F32, name="b_sb")
            warm_sb = persist.tile([128, 512], BF16, name="warm_sb")

            # --- input DMAs ------------------------------------------------
            # Queue FIFO order = priority order; each DMA_DIRECT2D costs
            # ~0.65us of issue time on its queue engine and completion sems
            # lag the data by ~1.5-2us (HBM write receipt). Critical path to
            # the first conv matmul: ab -> LoRA MMs -> (with wt q0) weff
            # fold 0; x0 rows chase the first wave's row-groups.
            qs = [nc.sync, nc.scalar]
            wq = (9 * C_OUT) // 4  # wt quarter = 576 cols
            # Measured: each HW queue sustains only ~90 GB/s early and
            # completion sems lag data by ~1.7us, so the critical DMAs must
            # sit at the FRONT of their queue with nothing big ahead.
            #   sync:   ab (tiny, gates LoRA), wt q0, wt q1
            #   scalar: x0 in two wave-aligned chunks (rows 0-33 / 34-65)
            #   gpsimd (SWDGE): wt q2, wt q3, bias, then all of x1
            nc.sync.dma_start(ab_sb[:], ab)
            nc.sync.dma_start(wt_sb[:, 0:wq], wt[:, 0:wq])
            nc.sync.dma_start(wt_sb[:, wq : 2 * wq], wt[:, wq : 2 * wq])
            xa = 34 * WP  # rows 0..33 cover conv wave A (rg0-3)
            nc.scalar.dma_start(x_sb[0][:, :xa], xp[0, :, :xa])
            nc.scalar.dma_start(x_sb[0][:, xa:], xp[0, :, xa:])
            nc.gpsimd.dma_start(wt_sb[:, 2 * wq : 3 * wq], wt[:, 2 * wq : 3 * wq])
            nc.gpsimd.dma_start(wt_sb[:, 3 * wq :], wt[:, 3 * wq :])
            nc.gpsimd.dma_start(b_sb[:], bv)
            nc.gpsimd.dma_start(x_sb[1][:, :xa], xp[1, :, :xa])
            nc.gpsimd.dma_start(x_sb[1][:, xa:], xp[1, :, xa:])

            # --- PE warm-up ------------------------------------------------
            # The HAM clock gate holds the PE at 1.2 GHz until ~3.4us of
            # sustained busy. DVE is idle at the head, so it memsets the
            # warm tile immediately and the dummy matmuls (no DMA deps)
            # start the busy window at ~first-issue time.
            nc.vector.memset(warm_sb[:], 0.0)
            lps = [
                psum.tile([128, 512], F32, tag="lps", bufs=3, name=f"lps{j}")
                for j in range(5)
            ]
            for _ in range(3):
                nc.tensor.matmul(
                    lps[0][:], warm_sb[:, :128], warm_sb[:], start=True, stop=True
                )

            # --- fold LoRA into the conv weight ----------------------------
            # lps[j][:, (k%2)*256:...] = (A_k)^T @ B^T  for k = 2j, 2j+1
            # weff[:, k*256+co] = wt[:, k*256+co] + 2 * lps[...]
            # The 9 K=8 matmuls also extend the PE warm-up.
            for k in range(9):
                nc.tensor.matmul(
                    lps[k // 2][:, (k % 2) * 256 : (k % 2) * 256 + 256],
                    ab_sb[:, k * 128 : (k + 1) * 128],
                    ab_sb[:, 9 * C_IN :],
                    start=True,
                    stop=True,
                )


            # DVE stream is FIFO and the scheduler's DMA-latency model is
            # optimistic: chain the weff folds in k order so conv wave 0
            # starts progressively off fold #0.
            def chain(inst, prev, why):
                if prev is not None:
                    add_dep_helper(inst.ins, prev.ins, sync=False, reason=why)
                return inst

            link = None
            for j in range(5):
                w = 512 if j < 4 else 256
                link = chain(
                    nc.vector.scalar_tensor_tensor(
                        weff[:, j * 512 : j * 512 + w],
                        lps[j][:, :w],
                        SCALING,
                        wt_sb[:, j * 512 : j * 512 + w],
                        op0=mybir.AluOpType.mult,
                        op1=mybir.AluOpType.add,
                    ),
                    link,
                    "weff fold k order",
                )

            # --- the conv: 9 accumulating shift-matmuls per output tile ----
            # Emitted k-minor in 4-row-group waves: the in-order PE queue
            # then needs weff fold j only ~8 matmuls after fold j-1, so it
            # chases the DVE chain without stalling, and each weight is
            # loaded once per wave instead of once per tile.
            for img in range(B_LOC):
                x_r = x_sb[img][:].rearrange("p (h w) -> p h w", w=WP)
                for cb in range(2):
                    for wv in range(2):
                        rgs = [wv * 4 + j for j in range(4)]
                        ps = {
                            rg: psum.tile(
                                [128, 512], F32, tag="ps", bufs=5,
                                name=f"ps{img}_{cb}_{rg}",
                            )
                            for rg in rgs
                        }
                        for k in range(9):
                            dh, dw = k // 3 - 1, k % 3 - 1
                            lhsT = weff[
                                :, k * 256 + cb * 128 : k * 256 + cb * 128 + 128
                            ]
                            for rg in rgs:
                                h0 = rg * ROWS_PER_TILE
                                rhs = x_r[
                                    :,
                                    h0 + 1 + dh : h0 + 1 + dh + ROWS_PER_TILE,
                                    1 + dw : 65 + dw,
                                ]
                                nc.tensor.matmul(
                                    ps[rg][:],
                                    lhsT,
                                    rhs,
                                    start=(k == 0),
                                    stop=(k == 8),
                                )
                        for rg in rgs:
                            o = outp.tile(
                                [128, 512], BF16, tag="o", name=f"o{img}_{cb}_{rg}"
                            )
                            ti = (img * 2 + cb) * N_RG + rg
                            # Alternate the PSUM->SBUF bias-add between ACT
                            # and DVE so neither engine limits PSUM drain.
                            if ti % 2 == 0:
                                nc.scalar.activation(
                                    o[:], ps[rg][:], IDENT, bias=b_sb[:, cb : cb + 1]
                                )
                            else:
                                nc.vector.tensor_scalar_add(
                                    o[:], ps[rg][:], b_sb[:, cb : cb + 1]
                                )
                            dst = out[
                                img,
                                cb * 128 : (cb + 1) * 128,
                                rg * 512 : (rg + 1) * 512,
                            ]
                            # last two tiles land on different queues (ti
                            # parity), so they drain in parallel; splitting
                            # them further only clogs the queue with issue
                            # time (~0.65us per DMA instruction).
                            qs[ti % 2].dma_start(dst, o[:])

    nc.compile()
    return nc


_NC_CACHE = None


def _get_nc():
    global _NC_CACHE
    if _NC_CACHE is None:
        _NC_CACHE = _build_nc()
    return _NC_CACHE


def _host_prep(x, W, b, lora_A, lora_B):
    """Layout + bf16 rounding on host (identical RNE rounding to the DVE
    casts the kernel previously performed on device); no other arithmetic."""
    x = np.ascontiguousarray(x, dtype=np.float32)
    xp_all = np.zeros((B, C_IN, HP, WP), dtype=np.float32)
    xp_all[:, :, 1 : H + 1, 1 : W_DIM + 1] = x
    xp_all = xp_all.reshape(B, C_IN, HP * WP).astype(BF16_NP)

    # [co, ci, kh, kw] -> [ci, k, co]
    wt = (
        np.ascontiguousarray(
            np.asarray(W, dtype=np.float32).reshape(C_OUT, C_IN, 9).transpose(1, 2, 0)
        )
        .reshape(C_IN, 9 * C_OUT)
        .astype(BF16_NP)
    )
    # lora_A [r, ci*9+k] -> [r, k, ci]; lora_B [co, r] -> [r, co]; bundled
    at = np.asarray(lora_A, dtype=np.float32).reshape(RANK, C_IN, 9).transpose(0, 2, 1)
    bt = np.asarray(lora_B, dtype=np.float32).T
    ab = np.concatenate(
        [at.reshape(RANK, 9 * C_IN), bt], axis=1
    ).astype(BF16_NP)
    ab = np.ascontiguousarray(ab)
    # [256] -> [128, 2]: bv[p, cb] = b[cb*128 + p]
    bv = np.ascontiguousarray(np.asarray(b, dtype=np.float32).reshape(2, 128).T)
    return xp_all, wt, ab, bv


def run(x, W, b, lora_A, lora_B, trace=False):
    """Run the kernel on 8 cores; returns (full_output, BassKernelResults)."""
    xp_all, wt, ab, bv = _host_prep(x, W, b, lora_A, lora_B)
    nc = _get_nc()
    in_maps = []
    for c in range(N_CORES):
        in_maps.append(
            {
                "xp": np.ascontiguousarray(xp_all[c * B_LOC : (c + 1) * B_LOC]),
                "wt": wt,
                "ab": ab,
                "bv": bv,
            }
        )
    res = run_bass_kernel_spmd(
        nc, in_maps, core_ids=list(range(N_CORES)), trace=trace
    )
    out = np.concatenate(
        [r["out"].astype(np.float32) for r in res.results], axis=0
    )
    return out.reshape(B, C_OUT, H, W_DIM), res


def kernel(x, W, b, lora_A, lora_B):
    out, _ = run(x, W, b, lora_A, lora_B, trace=False)
    return out


# revision 12
# speedup vs baseline: 1.0234x; 1.0220x over previous
"""Conv2d(128->256, 3x3, pad 1) with LoRA (rank 8) — Trainium2 Bass kernel.

Strategy (v3):
  - Data-parallel over batch: 16 images -> 2 per core x 8 cores. Conv weights
    and LoRA A/B replicated.
  - LoRA folds into the conv weight on device (conv is linear in weights):
        W_eff = W + (alpha/rank) * (B @ A).reshape(C_OUT, C_IN, 3, 3)
    via 9 tiny PE matmuls (K=8) + fused DVE scalar_tensor_tensor adds.
  - The 3x3 conv itself = 9 shifted matmuls accumulating in PSUM:
        out[co, pix] += W_eff[co, :, kh, kw]^T @ x_shift[ci, pix]
    with K = C_IN = 128 (partition dim), M = 128 (co block), N = 512
    (8 image rows x 64 cols) in bf16 — 1 col/cycle = full PE rate; the
    288-matmul stream is the bf16 roofline (~62 us warm).
  - All tensor I/O is bf16 (host does the identical RNE rounding the DVE
    used to do; output is written bf16 and upcast on host). Halves DMA
    traffic, removes every DVE cast.
  - Head: A/B land first (one bundled DMA) -> LoRA matmuls double as PE
    warm-up; wt arrives in quarters on both HW queues; the conv is emitted
    in k-minor 4-row-group waves so the in-order PE queue chases the weff
    folds without stalling.
  - x image 1 + bias stream via the gpsimd SWDGE queue, keeping both HWDGE
    queues free for x0/wt in and output tiles out.
"""

import numpy as np
import ml_dtypes

import concourse.bass as bass
import concourse.tile as tile
from concourse.tile import add_dep_helper
from concourse import bacc, mybir
from concourse.bass_utils import run_bass_kernel_spmd

N_CORES = 8
B, C_IN, H, W_DIM = 16, 128, 64, 64
C_OUT = 256
RANK = 8
SCALING = 2.0  # alpha/rank = 16/8
HP, WP = H + 2, W_DIM + 2  # zero-padded image dims
B_LOC = B // N_CORES  # images per core
NPIX = H * W_DIM  # 4096
ROWS_PER_TILE = 8  # output rows per matmul group -> N = 8*64 = 512
N_RG = H // ROWS_PER_TILE  # 8 row groups

F32 = mybir.dt.float32
BF16 = mybir.dt.bfloat16
IDENT = mybir.ActivationFunctionType.Identity
BF16_NP = ml_dtypes.bfloat16


def _build_nc():
    nc = bacc.Bacc(
        "TRN2",
        target_bir_lowering=False,
        debug=False,
        num_devices=N_CORES,
    )

    xp = nc.dram_tensor("xp", [B_LOC, C_IN, HP * WP], BF16, kind="ExternalInput").ap()
    wt = nc.dram_tensor("wt", [C_IN, 9 * C_OUT], BF16, kind="ExternalInput").ap()
    # at and bt bundled: [8, 9*128 | 256] -> one DMA, one completion
    ab = nc.dram_tensor("ab", [RANK, 9 * C_IN + C_OUT], BF16, kind="ExternalInput").ap()
    bv = nc.dram_tensor("bv", [128, 2], F32, kind="ExternalInput").ap()
    out = nc.dram_tensor("out", [B_LOC, C_OUT, NPIX], BF16, kind="ExternalOutput").ap()

    with tile.TileContext(nc) as tc:
        with (
            tc.tile_pool(name="persist", bufs=1) as persist,
            tc.tile_pool(name="outp", bufs=6) as outp,
            tc.tile_pool(name="psum", bufs=8, space="PSUM") as psum,
        ):
            # --- persistent SBUF tiles (all bf16 straight off DMA) ----------
            x_sb = [
                persist.tile([C_IN, HP * WP], BF16, name=f"x_sb{i}")
                for i in range(B_LOC)
            ]
            wt_sb = persist.tile([C_IN, 9 * C_OUT], BF16, name="wt_sb")
            weff = persist.tile([C_IN, 9 * C_OUT], BF16, name="weff")
            ab_sb = persist.tile([RANK, 9 * C_IN + C_OUT], BF16, name="ab_sb")
            b_sb = persist.tile([128, 2], F32, name="b_sb")
            warm_sb = persist.tile([128, 512], BF16, name="warm_sb")

            # --- input DMAs ------------------------------------------------
            # Queue FIFO order = priority order; each DMA_DIRECT2D costs
            # ~0.65us of issue time on its queue engine and completion sems
            # lag the data by ~1.5-2us (HBM write receipt). Critical path to
            # the first conv matmul: ab -> LoRA MMs -> (with wt q0) weff
            # fold 0; x0 rows chase the first wave's row-groups.
            qs = [nc.sync, nc.scalar]
            # Measured: each queue sustains only ~65-90 GB/s early and
            # completion sems lag data by ~1.5-2.5us, so the critical DMAs
            # sit at the FRONT of their queues and wt arrives in five
            # 512-col pieces (one per weff fold, in fold order) spread over
            # all three queues so the folds pace ~evenly.
            #   sync:   ab (gates LoRA), wt p0, wt p1
            #   scalar: x0 wave A rows (gates conv rg0-3), wt p2, x0 wave B
            #   gpsimd (SWDGE): wt p3, wt p4, bias, x1
            nc.sync.dma_start(ab_sb[:], ab)
            nc.sync.dma_start(wt_sb[:, 0:512], wt[:, 0:512])
            nc.sync.dma_start(wt_sb[:, 512:1024], wt[:, 512:1024])
            xa = 34 * WP  # rows 0..33 cover conv wave A (rg0-3)
            nc.scalar.dma_start(x_sb[0][:, :xa], xp[0, :, :xa])
            nc.scalar.dma_start(wt_sb[:, 1024:1536], wt[:, 1024:1536])
            nc.scalar.dma_start(x_sb[0][:, xa:], xp[0, :, xa:])
            nc.gpsimd.dma_start(wt_sb[:, 1536:2048], wt[:, 1536:2048])
            nc.gpsimd.dma_start(wt_sb[:, 2048:], wt[:, 2048:])
            nc.gpsimd.dma_start(b_sb[:], bv)
            nc.gpsimd.dma_start(x_sb[1][:], xp[1, :, :])

            # --- PE warm-up ------------------------------------------------
            # The HAM clock gate holds the PE at 1.2 GHz until ~3.4us of
            # sustained busy. DVE is idle at the head, so it memsets the
            # warm tile immediately and the dummy matmuls (no DMA deps)
            # start the busy window at ~first-issue time.
            nc.vector.memset(warm_sb[:], 0.0)
            lps = [
                psum.tile([128, 512], F32, tag="lps", bufs=3, name=f"lps{j}")
                for j in range(5)
            ]
            for _ in range(3):
                nc.tensor.matmul(
                    lps[0][:], warm_sb[:, :128], warm_sb[:], start=True, stop=True
                )

            # --- fold LoRA into the conv weight ----------------------------
            # lps[j][:, (k%2)*256:...] = (A_k)^T @ B^T  for k = 2j, 2j+1
            # weff[:, k*256+co] = wt[:, k*256+co] + 2 * lps[...]
            # The K=8 matmuls also extend the PE warm-up. All 9 MUST be
            # emitted before the folds: Tile dependency tracking is
            # backward-looking, so a fold emitted before its lps writer
            # would silently read a stale bank (k6..k8 briefly cost a bank
            # wait on fold A freeing lps[0], ~1.3us on the PE queue).
            for k in range(9):
                nc.tensor.matmul(
                    lps[k // 2][:, (k % 2) * 256 : (k % 2) * 256 + 256],
                    ab_sb[:, k * 128 : (k + 1) * 128],
                    ab_sb[:, 9 * C_IN :],
                    start=True,
                    stop=True,
                )


            # DVE stream is FIFO and the scheduler's DMA-latency model is
            # optimistic: chain the weff folds in k order so conv wave 0
            # starts progressively off fold #0.
            def chain(inst, prev, why):
                if prev is not None:
                    add_dep_helper(inst.ins, prev.ins, sync=False, reason=why)
                return inst

            link = None
            for j in range(5):
                w = 512 if j < 4 else 256
                link = chain(
                    nc.vector.scalar_tensor_tensor(
                        weff[:, j * 512 : j * 512 + w],
                        lps[j][:, :w],
                        SCALING,
                        wt_sb[:, j * 512 : j * 512 + w],
                        op0=mybir.AluOpType.mult,
                        op1=mybir.AluOpType.add,
                    ),
                    link,
                    "weff fold k order",
                )

            # --- the conv: 9 accumulating shift-matmuls per output tile ----
            # Emitted k-minor in 4-row-group waves: the in-order PE queue
            # then needs weff fold j only ~8 matmuls after fold j-1, so it
            # chases the DVE chain without stalling, and each weight is
            # loaded once per wave instead of once per tile.
            first_wave = True
            for img in range(B_LOC):
                x_r = x_sb[img][:].rearrange("p (h w) -> p h w", w=WP)
                for cb in range(2):
                    for wv in range(2):
                        rgs = [wv * 4 + j for j in range(4)]
                        ps = {
                            rg: psum.tile(
                                [128, 512], F32, tag="ps", bufs=5,
                                name=f"ps{img}_{cb}_{rg}",
                            )
                            for rg in rgs
                        }
                        for k in range(9):
                            dh, dw = k // 3 - 1, k % 3 - 1
                            lhsT = weff[
                                :, k * 256 + cb * 128 : k * 256 + cb * 128 + 128
                            ]
                            for rg in rgs:
                                h0 = rg * ROWS_PER_TILE
                                rhs = x_r[
                                    :,
                                    h0 + 1 + dh : h0 + 1 + dh + ROWS_PER_TILE,
                                    1 + dw : 65 + dw,
                                ]
                                nc.tensor.matmul(
                                    ps[rg][:],
                                    lhsT,
                                    rhs,
                                    start=(k == 0),
                                    stop=(k == 8),
                                )

                        # drain: PSUM -> bf16 SBUF (+bias) per row group,
                        # alternating ACT/DVE; one out-DMA per rg PAIR
                        # (fewer DMA instructions and semaphores -> shorter
                        # issue queues and end-of-kernel sem-clear parade).
                        for pr in range(2):
                            rga, rgb = rgs[2 * pr], rgs[2 * pr + 1]
                            o = outp.tile(
                                [128, 1024], BF16, tag="o", name=f"o{img}_{cb}_{rga}"
                            )
                            ti = (img * 2 + cb) * N_RG + rga
                            for h, rg in ((0, rga), (1, rgb)):
                                if (ti + h) % 2 == 0:
                                    nc.scalar.activation(
                                        o[:, h * 512 : h * 512 + 512],
                                        ps[rg][:],
                                        IDENT,
                                        bias=b_sb[:, cb : cb + 1],
                                    )
                                else:
                                    nc.vector.tensor_scalar_add(
                                        o[:, h * 512 : h * 512 + 512],
                                        ps[rg][:],
                                        b_sb[:, cb : cb + 1],
                                    )
                            dst = out[
                                img,
                                cb * 128 : (cb + 1) * 128,
                                rga * 512 : rga * 512 + 1024,
                            ]
                            qs[(ti // 2) % 2].dma_start(dst, o[:])
                        first_wave = False

    nc.compile()
    return nc


_NC_CACHE = None


def _get_nc():
    global _NC_CACHE
    if _NC_CACHE is None:
        _NC_CACHE = _build_nc()
    return _NC_CACHE


def _host_prep(x, W, b, lora_A, lora_B):
    """Layout + bf16 rounding on host (identical RNE rounding to the DVE
    casts the kernel previously performed on device); no other arithmetic."""
    x = np.ascontiguousarray(x, dtype=np.float32)
    xp_all = np.zeros((B, C_IN, HP, WP), dtype=np.float32)
    xp_all[:, :, 1 : H + 1, 1 : W_DIM + 1] = x
    xp_all = xp_all.reshape(B, C_IN, HP * WP).astype(BF16_NP)

    # [co, ci, kh, kw] -> [ci, k, co]
    wt = (
        np.ascontiguousarray(
            np.asarray(W, dtype=np.float32).reshape(C_OUT, C_IN, 9).transpose(1, 2, 0)
        )
        .reshape(C_IN, 9 * C_OUT)
        .astype(BF16_NP)
    )
    # lora_A [r, ci*9+k] -> [r, k, ci]; lora_B [co, r] -> [r, co]; bundled
    at = np.asarray(lora_A, dtype=np.float32).reshape(RANK, C_IN, 9).transpose(0, 2, 1)
    bt = np.asarray(lora_B, dtype=np.float32).T
    ab = np.concatenate(
        [at.reshape(RANK, 9 * C_IN), bt], axis=1
    ).astype(BF16_NP)
    ab = np.ascontiguousarray(ab)
    # [256] -> [128, 2]: bv[p, cb] = b[cb*128 + p]
    bv = np.ascontiguousarray(np.asarray(b, dtype=np.float32).reshape(2, 128).T)
    return xp_all, wt, ab, bv


def run(x, W, b, lora_A, lora_B, trace=False):
    """Run the kernel on 8 cores; returns (full_output, BassKernelResults)."""
    xp_all, wt, ab, bv = _host_prep(x, W, b, lora_A, lora_B)
    nc = _get_nc()
    in_maps = []
    for c in range(N_CORES):
        in_maps.append(
            {
                "xp": np.ascontiguousarray(xp_all[c * B_LOC : (c + 1) * B_LOC]),
                "wt": wt,
                "ab": ab,
                "bv": bv,
            }
        )
    res = run_bass_kernel_spmd(
        nc, in_maps, core_ids=list(range(N_CORES)), trace=trace
    )
    out = np.concatenate(
        [r["out"].astype(np.float32) for r in res.results], axis=0
    )
    return out.reshape(B, C_OUT, H, W_DIM), res


def kernel(x, W, b, lora_A, lora_B):
    out, _ = run(x, W, b, lora_A, lora_B, trace=False)
    return out
